# revision 1
# baseline (speedup 1.0000x reference)
"""Trainium2 Bass kernel for nn_CrossAttention_38783554683555.

Two cross-attention branches x 4 batches = 8 independent attention units,
one per NeuronCore (pure data parallel, no collectives).

Per core (N=4096, C=64, D=16):
  q_in [D,N] = Wf@feat + Wo@other + b      (scale folded host-side)
  k    [D,N] = Wk@feat + bk
  V'^T [N,C+1] = [feat^T @ Wv^T + bv | 1]  (ones col -> softmax denom for free)
  S^T[n,m] = k[:,n].q_in[:,m]   (keys on partitions -> row softmax = partition sum)
  E = exp(S^T)                  (no max subtraction; logits are tiny)
  av[c',m] = sum_n V'^T[n,c'] E[n,m]  ; row C = denominator
  out = gamma*av[0:C]/av[C] + feat
"""

import numpy as np

import concourse.bass as bass
import concourse.tile as tile
from concourse import bacc, mybir
from concourse.bass_utils import run_bass_kernel_spmd

N = 4096          # H*W
C = 64            # channels
D = 16            # query/key dim after concat
MB = 1024         # m-block (query) size
NMB = N // MB     # 4
NCH = N // 128    # 32 key chunks
B = 4
SCALE = (C // 8) ** -0.5
VC = 66       # V' columns (64 ch + denom ones col + even-size pad)

F32 = mybir.dt.float32
F32R = mybir.dt.float32r
BF16 = mybir.dt.bfloat16
EXP = mybir.ActivationFunctionType.Exp

_CACHE = {}


def build_nc():
    nc = bacc.Bacc("TRN2", target_bir_lowering=False)
    featE = nc.dram_tensor("featE", [C + 1, N], F32R, kind="ExternalInput")
    other = nc.dram_tensor("other", [C, N], F32R, kind="ExternalInput")
    wqf = nc.dram_tensor("wqf", [C + 1, D], F32R, kind="ExternalInput")
    wqo = nc.dram_tensor("wqo", [C, D], F32R, kind="ExternalInput")
    wkE = nc.dram_tensor("wkE", [C + 1, D], F32R, kind="ExternalInput")
    wvE = nc.dram_tensor("wvE", [C + 1, VC], F32R, kind="ExternalInput")
    onesg = nc.dram_tensor("onesg", [1, C], BF16, kind="ExternalInput")
    out = nc.dram_tensor("out", [C, N], F32, kind="ExternalOutput")

    with tile.TileContext(nc) as tc:
        with (
            tc.tile_pool(name="const", bufs=1) as cpool,
            tc.tile_pool(name="epool", bufs=6) as epool,
            tc.tile_pool(name="tail", bufs=3) as tpool,
            tc.tile_pool(name="spsum", bufs=2, space="PSUM") as spool,
            tc.tile_pool(name="avpsum", bufs=2, space="PSUM") as avpool,
        ):
            feh = [cpool.tile([C + 1, N // 4], F32R, name=f"feh{i}")
                   for i in range(4)]
            oth = [cpool.tile([C, N // 4], F32R, name=f"oth{i}")
                   for i in range(4)]
            wqf_s = cpool.tile([C + 1, D], F32R)
            wqo_s = cpool.tile([C, D], F32R)
            wk_s = cpool.tile([C + 1, D], F32R)
            wv_s = cpool.tile([C + 1, VC], F32R)
            og_s = cpool.tile([1, C], BF16)
            qin = cpool.tile([D, N], F32R)
            kk = cpool.tile([D, N], F32R)
            vT = cpool.tile([128, VC * NCH], F32R)

            def fe_sl(lo, sz):
                h, off = divmod(lo, N // 4)
                assert off + sz <= N // 4
                return feh[h][:, off:off + sz]

            def ot_sl(lo, sz):
                h, off = divmod(lo, N // 4)
                return oth[h][:, off:off + sz]

            nc.gpsimd.dma_start(wqf_s[:], wqf[:])
            nc.gpsimd.dma_start(wqo_s[:], wqo[:])
            nc.gpsimd.dma_start(wk_s[:], wkE[:])
            nc.gpsimd.dma_start(wv_s[:], wvE[:])
            nc.gpsimd.dma_start(og_s[:], onesg[:])
            for i in range(4):
                sl = slice(i * (N // 4), (i + 1) * (N // 4))
                nc.sync.dma_start(feh[i][:], featE[:, sl])
                nc.gpsimd.dma_start(oth[i][:], other[:, sl])

            # ---- prologue slab emitters (interleaved into the flash loop) ----
            def emit_qin(j, pool=None, tg="s", eng=None):
                ps = (pool or spool).tile([128, MB], F32, tag=tg, name=f"psq{j}")
                for h in range(2):
                    sl = slice(h * 512, (h + 1) * 512)
                    col = j * MB + h * 512
                    nc.tensor.matmul(ps[0:D, sl], wqf_s[:], fe_sl(col, 512),
                                     start=True, stop=False)
                    nc.tensor.matmul(ps[0:D, sl], wqo_s[:], ot_sl(col, 512),
                                     start=False, stop=True)
                dst = qin[:, j * MB:(j + 1) * MB]
                if eng is nc.scalar:
                    nc.scalar.copy(dst, ps[0:D, :])
                else:
                    nc.vector.tensor_copy(dst, ps[0:D, :])

            def emit_kk(j, pool=None, tg="s", eng=None):
                ps = (pool or spool).tile([128, MB], F32, tag=tg, name=f"psk{j}")
                for h in range(2):
                    sl = slice(h * 512, (h + 1) * 512)
                    col = j * MB + h * 512
                    nc.tensor.matmul(ps[0:D, sl], wk_s[:], fe_sl(col, 512),
                                     start=True, stop=True)
                dst = kk[:, j * MB:(j + 1) * MB]
                if eng is nc.scalar:
                    nc.scalar.copy(dst, ps[0:D, :])
                else:
                    nc.vector.tensor_copy(dst, ps[0:D, :])

            def emit_vt(g, pool=None, tg="s"):
                ps = (pool or spool).tile([128, MB], F32, tag=tg, name=f"psv{g}")
                for t in range(4):
                    c = g * 4 + t
                    nc.tensor.matmul(ps[:, t * VC:(t + 1) * VC],
                                     fe_sl(c * 128, 128), wv_s[:],
                                     start=True, stop=True)
                nc.vector.tensor_copy(vT[:, g * 4 * VC:(g + 1) * 4 * VC],
                                      ps[:, 0:4 * VC])

            # minimum needed before the flash loop can start; use the av
            # pool's slots (idle until AV(0,0)) so the s pool is free for
            # the first S slabs
            emit_kk(0, avpool, "av")
            emit_qin(0, avpool, "av")
            emit_vt(0, avpool, "av")
            # remaining prologue work, emitted during mb0 chunks (key: chunk c)
            AV, A = avpool, "av"
            deferred = {(0, 0): [lambda: emit_vt(1, AV, A)],
                        (0, 4): [lambda: emit_vt(2, AV, A)],
                        (0, 5): [lambda: emit_kk(1, AV, A)],
                        (0, 8): [lambda: emit_vt(3, AV, A)],
                        (0, 12): [lambda: emit_vt(4, AV, A)],
                        (0, 13): [lambda: emit_kk(2, AV, A)],
                        (0, 16): [lambda: emit_vt(5, AV, A)],
                        (0, 20): [lambda: emit_vt(6, AV, A)],
                        (0, 21): [lambda: emit_kk(3, AV, A)],
                        (0, 24): [lambda: emit_vt(7, AV, A)],
                        (0, 26): [lambda: emit_qin(1, AV, A)],
                        (1, 10): [lambda: emit_qin(2, AV, A)],
                        (2, 10): [lambda: emit_qin(3, AV, A)]}

            # ---- flash loop, software-pipelined so PE never waits on ACT ----
            av_tiles = {}

            def emit_av(mb, c):
                if mb not in av_tiles:
                    av_tiles[mb] = avpool.tile([128, MB], F32, tag="av",
                                               name=f"av{mb}")
                av = av_tiles[mb]
                e = e_tiles.pop((mb, c))
                vt = vT[:, c * VC:(c + 1) * VC]
                for h in range(2):
                    sl = slice(h * 512, (h + 1) * 512)
                    nc.tensor.matmul(av[0:VC, sl], vt, e[:, sl],
                                     start=(c == 0), stop=(c == NCH - 1))

            def emit_tail(mb):
                # half-width chains so the h0 tail overlaps h1's last AV and
                # the h0 store overlaps the h1 compute
                av = av_tiles.pop(mb)
                rc = tpool.tile([1, MB], BF16, tag="rc")
                rb = tpool.tile([C, MB], F32, tag="rb")
                o1 = tpool.tile([C, MB], F32, tag="o1")
                o2 = tpool.tile([C, MB], F32, tag="o2")
                fes = fe_sl(mb * MB, MB)
                for h in range(2):
                    sl = slice(h * 512, (h + 1) * 512)
                    with nc.allow_low_precision(reason="denom fits bf16"):
                        nc.vector.reciprocal(rc[:, sl], av[C:C + 1, sl])
                    # broadcast gamma/denom across partitions via K=1 bf16
                    # matmul into the unused upper partitions of the av tile
                    nc.tensor.matmul(av[C:C + C, sl], og_s[:], rc[:, sl],
                                     start=True, stop=True)
                    nc.vector.tensor_copy(rb[:, sl], av[C:C + C, sl])
                    nc.vector.tensor_mul(o1[:, sl], av[0:C, sl], rb[:, sl])
                    nc.vector.tensor_add(o2[:, sl], o1[:, sl], fes[0:C, sl])
                    nc.sync.dma_start(out[:, mb * MB + h * 512:
                                          mb * MB + (h + 1) * 512],
                                      o2[:, sl])

            e_tiles = {}
            prev = None
            for mb in range(NMB):
                for c in range(NCH):
                    s = spool.tile([128, MB], F32, tag="s")
                    kt = kk[:, c * 128:(c + 1) * 128]
                    for h in range(2):
                        sl = slice(h * 512, (h + 1) * 512)
                        nc.tensor.matmul(s[:, sl], kt,
                                         qin[:, mb * MB + h * 512: mb * MB + (h + 1) * 512],
                                         start=True, stop=True)
                    e = epool.tile([128, MB], F32R, tag="e")
                    nc.scalar.activation(e[:], s[:], EXP)
                    e_tiles[(mb, c)] = e
                    if prev is not None:
                        emit_av(*prev)
                    for fn in deferred.pop((mb, c), []):
                        fn()
                    if c == 4 and mb > 0:
                        emit_tail(mb - 1)
                    prev = (mb, c)
            emit_av(*prev)
            emit_tail(NMB - 1)

    nc.compile()
    return nc


def _prep_core_inputs(inputs):
    """Build the 8 per-core input maps (host-side weight folding)."""
    x1 = np.asarray(inputs["input1"], np.float32).reshape(B, C, N)
    x2 = np.asarray(inputs["input2"], np.float32).reshape(B, C, N)
    g = lambda k: np.asarray(inputs[k], np.float32)
    wq = [g("wq1"), g("wq2"), g("wq3"), g("wq4")]
    bq = [g("bq1"), g("bq2"), g("bq3"), g("bq4")]
    Z = np.zeros_like(wq[0])
    gamma = float(np.asarray(inputs["gamma"]).reshape(-1)[0])

    # q_in1 = [q1, q3, q4, q2];  x3 = (2/3)x1+(1/3)x2, x4 = (1/3)x1+(2/3)x2
    Wf1 = np.vstack([wq[0], (2 / 3) * wq[2], (1 / 3) * wq[3], Z])
    Wo1 = np.vstack([Z, (1 / 3) * wq[2], (2 / 3) * wq[3], wq[1]])
    b1 = np.concatenate([bq[0], bq[2], bq[3], bq[1]])
    # q_in2 = [q2, q4, q3, q1]; feat = x2, other = x1
    Wf2 = np.vstack([wq[1], (2 / 3) * wq[3], (1 / 3) * wq[2], Z])
    Wo2 = np.vstack([Z, (1 / 3) * wq[3], (2 / 3) * wq[2], wq[0]])
    b2 = np.concatenate([bq[1], bq[3], bq[2], bq[0]])

    ones_row = np.ones((1, N), np.float32)
    import ml_dtypes
    onesg = np.full((1, C), gamma, ml_dtypes.bfloat16)

    def branch_weights(r):
        if r == 0:
            Wf, Wo, bb = Wf1, Wo1, b1
            wk_, bk_, wv_, bv_ = g("wk"), g("bk"), g("wv"), g("bv")
        else:
            Wf, Wo, bb = Wf2, Wo2, b2
            wk_, bk_, wv_, bv_ = g("wk2"), g("bk2"), g("wv2"), g("bv2")
        wqf = np.vstack([(SCALE * Wf).T, (SCALE * bb)[None, :]])          # [65,16]
        wqo = (SCALE * Wo).T                                              # [64,16]
        wkE = np.vstack([wk_.T, bk_[None, :]])                            # [65,16]
        wvE = np.zeros((C + 1, 66), np.float32)
        wvE[:C, :C] = wv_.T
        wvE[C, :C] = bv_
        wvE[C, C] = 1.0
        return wqf, wqo, wkE, wvE

    wsets = [branch_weights(0), branch_weights(1)]
    in_maps = []
    for core in range(8):
        r, b = divmod(core, B)
        feat = x1[b] if r == 0 else x2[b]
        other = x2[b] if r == 0 else x1[b]
        wqf, wqo, wkE, wvE = wsets[r]
        in_maps.append({
            "featE": np.ascontiguousarray(np.vstack([feat, ones_row])),
            "other": np.ascontiguousarray(other),
            "wqf": np.ascontiguousarray(wqf),
            "wqo": np.ascontiguousarray(wqo),
            "wkE": np.ascontiguousarray(wkE),
            "wvE": np.ascontiguousarray(wvE),
            "onesg": onesg,
        })
    return in_maps


def run(inputs, trace=False, **kw):
    if "nc" not in _CACHE:
        _CACHE["nc"] = build_nc()
    nc = _CACHE["nc"]
    in_maps = _prep_core_inputs(inputs)
    res = run_bass_kernel_spmd(nc, in_maps, list(range(8)), trace=trace, **kw)
    out1 = np.stack([res.results[b]["out"].reshape(C, 64, 64) for b in range(B)])
    out2 = np.stack([res.results[4 + b]["out"].reshape(C, 64, 64) for b in range(B)])
    return (out1, out2), res


def kernel(**inputs):
    (out1, out2), _ = run(inputs)
    return out1, out2

